# revision 26
# baseline (speedup 1.0000x reference)
"""Trainium2 Bass kernel for GaussianEmbeddingDP.

Reference computation (per row of a [16384, 1024] f32 tensor):
    norm  = ||row||_2
    scale = 1 / max(norm / 1.0, 1.0)
    out   = row * scale + 1.1 * N(0,1)  (noise from jax.random.key(42), fixed)

The noise tensor is a deterministic constant (fixed PRNG key), so it is
precomputed on host once and streamed to the device like a weight. The
device kernel does: row sum-of-squares -> clip-scale -> multiply -> add
noise. Pure data parallel over 8 NeuronCores (2048 rows each).
"""

import numpy as np

import concourse.bacc as bacc
import concourse.bass as bass
import concourse.mybir as mybir
import concourse.tile as tile
from concourse.bass_utils import run_bass_kernel_spmd

N_CORES = 8
B, D = 16384, 1024
ROWS_PER_CORE = B // N_CORES  # 2048
P = 128                       # SBUF partitions
G = 2                         # row-groups of 128 per SBUF tile
N_TILES = ROWS_PER_CORE // (P * G)

# tuning knobs (overridable for cost-model sweeps); cost-model sweep picked
# G=1 (512KB DMAs over 8 HWDGE queues), scale-mul on DVE, 4 io bufs:
# 69.9 us/rep marginal vs 68.4 us model DMA roofline.
CFG = {
    "G": 1,
    "io_bufs": 4,
    "sq_bufs": 2,
    "mul_engine": "dve",   # "act" | "dve"
    "nz_dma": "sync",      # engine for noise loads
    "out_dma": "gpsimd",   # engine for output stores (SWDGE lanes + its own
                           # issue sequencer; ~1.5us/rep faster on HW)
    "sq_bcast": False,     # write square-pass output to a stride-0 dummy
                           # (sq is never read; saves SBUF write traffic)
    "swq": 1,              # num_swdge_queues for gpsimd-issued DMAs
}

NOISE_SCALE = np.float32(1.1 * 1.0)  # NOISE_MULTIPLIER * L2_NORM_CLIP

_noise = None
_nc_cache = {}


def _get_noise() -> np.ndarray:
    """1.1 * jax.random.normal(key(42), (B, D), f32), computed on host CPU."""
    global _noise
    if _noise is None:
        import jax

        with jax.default_device(jax.devices("cpu")[0]):
            n = jax.random.normal(jax.random.key(42), (B, D), dtype=np.float32)
            _noise = np.asarray(n) * NOISE_SCALE
    return _noise


def _build_nc(loop: int = 1, unroll: int = 1) -> bass.Bass:
    """Build the per-core program. loop>1 wraps the whole kernel body in a
    device-side For_i loop (device-time measurement via wall-time slope);
    unroll>1 repeats the body inline (for TimelineSim marginal cost)."""
    cfg = dict(CFG)
    g_sz = cfg["G"]
    n_tiles = ROWS_PER_CORE // (P * g_sz)
    key = (loop, unroll, tuple(sorted(cfg.items())))
    if key in _nc_cache:
        return _nc_cache[key]

    nc = bacc.Bacc(
        "TRN2",
        target_bir_lowering=False,
        debug=False,
        num_swdge_queues=cfg["swq"],
    )
    f32 = mybir.dt.float32
    x = nc.declare_dram_parameter("x", [ROWS_PER_CORE, D], f32, isOutput=False)
    nz = nc.declare_dram_parameter("nz", [ROWS_PER_CORE, D], f32, isOutput=False)
    out = nc.declare_dram_parameter("out", [ROWS_PER_CORE, D], f32, isOutput=True)

    xt = x[:].rearrange("(n g p) d -> n p g d", p=P, g=g_sz)
    nzt = nz[:].rearrange("(n g p) d -> n p g d", p=P, g=g_sz)
    outt = out[:].rearrange("(n g p) d -> n p g d", p=P, g=g_sz)

    with tile.TileContext(nc) as tc:
        with (
            tc.tile_pool(name="io", bufs=cfg["io_bufs"]) as io,
            tc.tile_pool(name="sq", bufs=cfg["sq_bufs"]) as sqp,
            tc.tile_pool(name="stats", bufs=4) as stats,
        ):
            def body():
                for i in range(n_tiles):
                    x_tile = io.tile([P, g_sz, D], f32, tag="x")
                    nz_tile = io.tile([P, g_sz, D], f32, tag="nz")
                    if cfg["sq_bcast"]:
                        sq_tile = None
                        sq_dummy = sqp.tile([P, 1], f32, tag="sqd")
                    else:
                        sq_tile = sqp.tile([P, g_sz, D], f32, tag="sq")
                    nc.sync.dma_start(out=x_tile, in_=xt[i])
                    nz_eng = (
                        ("sync", "gpsimd")[i % 2]
                        if cfg["nz_dma"] == "alt"
                        else cfg["nz_dma"]
                    )
                    getattr(nc, nz_eng).dma_start(out=nz_tile, in_=nzt[i])

                    ss = stats.tile([P, g_sz], f32, tag="ss")
                    for g in range(g_sz):
                        # sq = x*x (scratch, never read), ss[:, g] = sum per
                        # row. (tensor_tensor_reduce would fuse this on DVE,
                        # but its raw-ISA encoding faults this runtime.)
                        sq_out = (
                            sq_dummy.broadcast_to(x_tile[:, g, :].shape)
                            if cfg["sq_bcast"]
                            else sq_tile[:, g, :]
                        )
                        nc.scalar.activation(
                            out=sq_out,
                            in_=x_tile[:, g, :],
                            func=mybir.ActivationFunctionType.Square,
                            accum_out=ss[:, g : g + 1],
                        )

                    # scale = 1 / sqrt(max(ss, 1))   (= 1/max(norm, 1))
                    scale_t = stats.tile([P, g_sz], f32, tag="scale")
                    nc.vector.tensor_scalar_max(out=ss, in0=ss, scalar1=1.0)
                    nc.scalar.sqrt(out=scale_t, in_=ss)
                    nc.vector.reciprocal(out=scale_t, in_=scale_t)

                    for g in range(g_sz):
                        # x *= scale (per-partition scalar broadcast)
                        if cfg["mul_engine"] == "act":
                            nc.scalar.activation(
                                out=x_tile[:, g, :],
                                in_=x_tile[:, g, :],
                                func=mybir.ActivationFunctionType.Copy,
                                bias=0.0,
                                scale=scale_t[:, g : g + 1],
                            )
                        else:
                            nc.vector.tensor_scalar_mul(
                                out=x_tile[:, g, :],
                                in0=x_tile[:, g, :],
                                scalar1=scale_t[:, g : g + 1],
                            )

                    nc.vector.tensor_add(out=x_tile, in0=x_tile, in1=nz_tile)
                    getattr(nc, cfg["out_dma"]).dma_start(
                        out=outt[i], in_=x_tile)

            if loop > 1:
                with tc.For_i(0, loop, 1):
                    for _ in range(unroll):
                        body()
            else:
                for _ in range(unroll):
                    body()

    nc.compile()
    _nc_cache[key] = nc
    return nc


def run(x_full: np.ndarray, trace: bool = False):
    """Shard, run on 8 cores, gather. Returns (out_full, BassKernelResults)."""
    x_full = np.ascontiguousarray(np.asarray(x_full, dtype=np.float32))
    assert x_full.shape == (B, D), x_full.shape
    noise = _get_noise()
    nc = _build_nc()
    in_maps = []
    for c in range(N_CORES):
        sl = slice(c * ROWS_PER_CORE, (c + 1) * ROWS_PER_CORE)
        in_maps.append(
            {"x": x_full[sl], "nz": np.ascontiguousarray(noise[sl])}
        )
    res = run_bass_kernel_spmd(
        nc, in_maps, core_ids=list(range(N_CORES)), trace=trace
    )
    out = np.concatenate([r["out"] for r in res.results], axis=0)
    return out, res


def kernel(**inputs) -> np.ndarray:
    out, _ = run(inputs["inputs"], trace=False)
    return out
